# revision 1
# baseline (speedup 1.0000x reference)
"""Trainium2 8-core kernel for nn_Attention_27530740367526.

Multi-head causal attention (B=2, S=2048, D=2048, H=16, HD=128, fp32) with
RoPE, sharded batch x head-group across 8 NeuronCores: core c handles batch
c//4 and heads [4*(c%4), 4*(c%4)+4).  Each core computes q/k/v projections
(+RoPE), attention for its heads, and the slice of the wo projection those
heads feed — a partial [S, D] output.  The host sums the 4 partials per
batch (the row-parallel wo "all-reduce" is a host-side unshard).

On-device everything lives in "transposed land": qT/kT are [head_dim, seq]
with head-dim on partitions, so scores come out transposed ([k, q]), the
softmax denominator is a ones-column matmul, and PV / wo consume natural
layouts with zero on-device transposes.  RoPE's rotate-half is a 128x128
permutation matmul on the PE.  All matmul operands are float32r (fp32
rounded to 11 explicit mantissa bits, pre-rounded on the host bit-exactly)
which runs at full PE rate.
"""

import sys

if "/opt/trn_rl_repo" not in sys.path:
    sys.path.insert(0, "/opt/trn_rl_repo")

import numpy as np

import concourse.bacc as bacc
import concourse.mybir as mybir
import concourse.tile as tile
from concourse.bass_utils import run_bass_kernel_spmd

F32 = mybir.dt.float32
F32R = mybir.dt.float32r
AF = mybir.ActivationFunctionType

N_HEADS = 16
N_CORES = 8
B, S, D = 2, 2048, 2048
HD = D // N_HEADS
H_LOC = N_HEADS // (N_CORES // B)  # 4 heads per core
HW = H_LOC * HD                    # 512 columns per core
SC = 512                           # seq chunk (matmul moving free dim)
P = 128


def _round_f32r(x: np.ndarray) -> np.ndarray:
    """Host-side fp32 -> float32r rounding (RNE to 11 explicit mantissa
    bits); bit-exact with the device DVE rounding."""
    xi = np.ascontiguousarray(x, dtype=np.float32).view(np.uint32)
    nbits = 12
    lo = np.uint32((1 << nbits) - 1)
    half = np.uint32(1 << (nbits - 1))
    rem = xi & lo
    up = (rem > half) | ((rem == half) & (((xi >> nbits) & 1) == 1))
    r = (xi & ~lo) + np.where(up, np.uint32(1 << nbits), np.uint32(0))
    return r.view(np.float32)


def _build_core_kernel(causal: bool):
    KO = D // P
    NQC = S // SC
    NST = S // P
    inv_sqrt_hd = 1.0 / float(np.sqrt(HD))

    nc = bacc.Bacc(None, target_bir_lowering=False)

    xT = nc.dram_tensor("xT", [D, S], F32R, kind="ExternalInput")
    wqkvT = nc.dram_tensor("wqkvT", [D, 3 * HW], F32R, kind="ExternalInput")
    woT = nc.dram_tensor("woT", [HW, D], F32R, kind="ExternalInput")
    cosT = nc.dram_tensor("cosT", [HD, S], F32, kind="ExternalInput")
    sinT = nc.dram_tensor("sinT", [HD, S], F32, kind="ExternalInput")
    PT = nc.dram_tensor("PT", [HD, HD], F32R, kind="ExternalInput")
    ones = nc.dram_tensor("ones", [P, 1], F32R, kind="ExternalInput")
    if causal:
        maskT = nc.dram_tensor("maskT", [SC, SC], F32, kind="ExternalInput")
    else:
        maskT = nc.dram_tensor("maskT", [S, S], F32, kind="ExternalInput")
    y = nc.dram_tensor("y", [S, D], F32, kind="ExternalOutput")

    xT_r = xT.rearrange("(ko ki) s -> ki ko s", ki=P)
    wqkvT_r = wqkvT.rearrange("(ko ki) c -> ki ko c", ki=P)
    woT_r = woT.rearrange("(h ki) d -> ki h d", ki=P)

    with tile.TileContext(nc) as tc:
        with (
            tc.tile_pool(name="persist", bufs=1) as persist,
            tc.tile_pool(name="dram", bufs=1, space="DRAM") as dram,
        ):
            pt_sb = persist.tile([P, HD], F32R)
            nc.sync.dma_start(pt_sb[:], PT[:])

            # DRAM staging for qT/kT (per head, [hd, s]) and v ([st, s128, hd])
            q_dram = dram.tile([H_LOC, P, S], F32R)
            k_dram = dram.tile([H_LOC, P, S], F32R)
            v_dram = dram.tile([H_LOC, NST, P, HD], F32R)

            # ================ Phase A+B: projections + RoPE ================
            with (
                tc.tile_pool(name="wqkv", bufs=1) as wpool,
                tc.tile_pool(name="xa", bufs=2) as xa,
                tc.tile_pool(name="projtmp", bufs=3) as projtmp,
                tc.tile_pool(name="projout", bufs=3) as projout,
                tc.tile_pool(name="cs", bufs=2) as cspool,
                tc.tile_pool(name="pps", bufs=2, space="PSUM") as pps,
                tc.tile_pool(name="rps", bufs=2, space="PSUM") as rps,
            ):
                w_sb = wpool.tile([P, KO, 3 * HW], F32R)
                nc.sync.dma_start(w_sb[:], wqkvT_r[:])

                for sc in range(NQC):
                    ssl = slice(sc * SC, (sc + 1) * SC)
                    xt = xa.tile([P, KO, SC], F32R, tag="xt")
                    nc.sync.dma_start(xt[:], xT_r[:, :, ssl])
                    cos_t = cspool.tile([P, SC], F32, tag="cos")
                    sin_t = cspool.tile([P, SC], F32, tag="sin")
                    nc.sync.dma_start(cos_t[:], cosT[:, ssl])
                    nc.sync.dma_start(sin_t[:], sinT[:, ssl])

                    # q/k projections + rope, per head
                    for h in range(H_LOC):
                        for base, dst in ((0, q_dram), (HW, k_dram)):
                            cols = slice(base + h * HD, base + (h + 1) * HD)
                            ps = pps.tile([P, SC], F32, tag="proj")
                            for ko in range(KO):
                                nc.tensor.matmul(
                                    ps[:],
                                    w_sb[:, ko, cols],
                                    xt[:, ko],
                                    start=(ko == 0),
                                    stop=(ko == KO - 1),
                                )
                            plain = projtmp.tile([P, SC], F32R, tag="plain")
                            nc.scalar.copy(plain[:], ps[:])
                            rot = rps.tile([P, SC], F32, tag="rot")
                            nc.tensor.matmul(rot[:], pt_sb[:], plain[:])
                            qk = projout.tile([P, SC], F32R, tag="qk")
                            nc.vector.tensor_mul(qk[:], plain[:], cos_t[:])
                            tmp2 = projtmp.tile([P, SC], F32, tag="tmp2")
                            nc.vector.tensor_mul(tmp2[:], rot[:], sin_t[:])
                            nc.vector.tensor_add(qk[:], qk[:], tmp2[:])
                            nc.sync.dma_start(dst[h, :, ssl], qk[:])

                    # v projection (all heads at once), per 128-row s-tile
                    for sti in range(SC // P):
                        st = sc * (SC // P) + sti
                        lsl = slice(sti * P, (sti + 1) * P)
                        psv = pps.tile([P, HW], F32, tag="projv")
                        for ko in range(KO):
                            nc.tensor.matmul(
                                psv[:],
                                xt[:, ko, lsl],
                                w_sb[:, ko, 2 * HW : 3 * HW],
                                start=(ko == 0),
                                stop=(ko == KO - 1),
                            )
                        v_sb = projout.tile([P, HW], F32R, tag="v")
                        nc.scalar.copy(v_sb[:], psv[:])
                        for h in range(H_LOC):
                            nc.sync.dma_start(
                                v_dram[h, st],
                                v_sb[:, h * HD : (h + 1) * HD],
                            )

            # ============== Phase C+D: attention + wo projection ==============
            with (
                tc.tile_pool(name="attn_persist", bufs=1) as apersist,
            ):
                ones_sb = apersist.tile([P, 1], F32R)
                nc.sync.dma_start(ones_sb[:], ones[:])
                if causal:
                    nmask = SC // P
                    mask_sb = apersist.tile([P, nmask, SC], F32)
                    nc.sync.dma_start(
                        mask_sb[:], maskT.rearrange("(j ki) q -> ki j q", ki=P)
                    )
                # outT[h]: attention output, [hd, s] layout, f32r
                outT_sb = apersist.tile([P, H_LOC, S], F32R)

                with (
                    tc.tile_pool(name="qkv_h", bufs=2) as qkvp,
                    tc.tile_pool(name="exps", bufs=3) as expp,
                    tc.tile_pool(name="smax", bufs=2) as smaxp,
                    tc.tile_pool(name="genmask", bufs=3) as genmaskp,
                    tc.tile_pool(name="sps", bufs=2, space="PSUM") as sps,
                    tc.tile_pool(name="ops", bufs=2, space="PSUM") as ops,
                    tc.tile_pool(name="dps", bufs=2, space="PSUM") as dps,
                ):
                    for h in range(H_LOC):
                        qT_h = qkvp.tile([P, S], F32R, tag="qT")
                        kT_h = qkvp.tile([P, S], F32R, tag="kT")
                        v_h = qkvp.tile([P, NST, HD], F32R, tag="v")
                        nc.sync.dma_start(qT_h[:], q_dram[h])
                        nc.sync.dma_start(kT_h[:], k_dram[h])
                        nc.sync.dma_start(
                            v_h[:], v_dram[h].rearrange("st p c -> p st c")
                        )

                        for qc in range(NQC):
                            qsl = slice(qc * SC, (qc + 1) * SC)
                            nkb = (qc + 1) * (SC // P) if causal else NST
                            o_ps = ops.tile([P, SC], F32, tag="o")
                            d_ps = dps.tile([1, SC], F32, tag="d")
                            for kb in range(nkb):
                                s_ps = sps.tile([P, SC], F32, tag="s")
                                nc.tensor.matmul(
                                    s_ps[:],
                                    kT_h[:, kb * P : (kb + 1) * P],
                                    qT_h[:, qsl],
                                    skip_group_check=True,
                                )
                                if causal:
                                    j = kb - qc * (SC // P)
                                    if j >= 0:
                                        nc.vector.tensor_add(
                                            s_ps[:], s_ps[:], mask_sb[:, j]
                                        )
                                else:
                                    mt = genmaskp.tile([P, SC], F32, tag="mt")
                                    nc.sync.dma_start(
                                        mt[:],
                                        maskT[kb * P : (kb + 1) * P, qsl],
                                    )
                                    nc.vector.tensor_add(
                                        s_ps[:], s_ps[:], mt[:]
                                    )
                                e_sb = expp.tile([P, SC], F32R, tag="e")
                                nc.scalar.activation(
                                    e_sb[:], s_ps[:], AF.Exp, scale=inv_sqrt_hd
                                )
                                nc.tensor.matmul(
                                    o_ps[:],
                                    v_h[:, kb],
                                    e_sb[:],
                                    start=(kb == 0),
                                    stop=(kb == nkb - 1),
                                    skip_group_check=True,
                                )
                                nc.tensor.matmul(
                                    d_ps[:],
                                    ones_sb[:],
                                    e_sb[:],
                                    start=(kb == 0),
                                    stop=(kb == nkb - 1),
                                    skip_group_check=True,
                                )
                            recip = smaxp.tile([1, SC], F32, tag="recip")
                            nc.vector.reciprocal(recip[:], d_ps[:])
                            bcast = smaxp.tile([P, SC], F32, tag="bcast")
                            nc.gpsimd.partition_broadcast(bcast[:], recip[:])
                            nc.vector.tensor_mul(
                                outT_sb[:, h, qsl], o_ps[:], bcast[:]
                            )

                # -------- Phase D: output projection --------
                with (
                    tc.tile_pool(name="wo", bufs=1) as wop,
                    tc.tile_pool(name="yout", bufs=3) as youtp,
                    tc.tile_pool(name="yps", bufs=2, space="PSUM") as yps,
                ):
                    wo_sb = wop.tile([P, H_LOC, D], F32R)
                    nc.sync.dma_start(wo_sb[:], woT_r[:])
                    NDC = D // SC
                    for st in range(NST):
                        stsl = slice(st * P, (st + 1) * P)
                        for dc in range(NDC):
                            dsl = slice(dc * SC, (dc + 1) * SC)
                            y_ps = yps.tile([P, SC], F32, tag="y")
                            for h in range(H_LOC):
                                nc.tensor.matmul(
                                    y_ps[:],
                                    outT_sb[:, h, stsl],
                                    wo_sb[:, h, dsl],
                                    start=(h == 0),
                                    stop=(h == H_LOC - 1),
                                )
                            y_sb = youtp.tile([P, SC], F32, tag="ysb")
                            nc.vector.tensor_copy(y_sb[:], y_ps[:])
                            nc.sync.dma_start(y[stsl, dsl], y_sb[:])

    nc.compile()
    return nc


_NC_CACHE = {}


def _get_nc(causal: bool):
    if causal not in _NC_CACHE:
        _NC_CACHE[causal] = _build_core_kernel(causal)
    return _NC_CACHE[causal]


def _rope_perm_T() -> np.ndarray:
    # rotate_half as a matrix: (P_rh @ q)[d] = -q[d+HD/2] for d < HD/2,
    # q[d-HD/2] otherwise.  Returns P_rh.T for use as matmul lhsT.
    P_rh = np.zeros((HD, HD), dtype=np.float32)
    half = HD // 2
    for i in range(half):
        P_rh[i, half + i] = -1.0
        P_rh[half + i, i] = 1.0
    return np.ascontiguousarray(P_rh.T)


def _is_causal(m: np.ndarray) -> bool:
    tril = np.tril(np.ones((S, S), dtype=bool))
    if not np.all(m[tril] == 0.0):
        return False
    upper = m[~tril]
    return bool(upper.size == 0 or np.all(upper <= -1.0e8))


# module-level: results of the last traced run (for test harnesses)
last_exec_time_ns = None
last_profile_json = None


def kernel(x, cos, sin, mask, wq, wk, wv, wo, _trace=False):
    x = np.asarray(x, dtype=np.float32)
    cos = np.asarray(cos, dtype=np.float32)
    sin = np.asarray(sin, dtype=np.float32)
    mask = np.asarray(mask, dtype=np.float32)
    wq = np.asarray(wq, dtype=np.float32)
    wk = np.asarray(wk, dtype=np.float32)
    wv = np.asarray(wv, dtype=np.float32)
    wo = np.asarray(wo, dtype=np.float32)

    m2d = mask.reshape(S, S)
    causal = _is_causal(m2d)
    nc = _get_nc(causal)

    scale = np.float32(np.sqrt(HD))
    if causal:
        maskT = np.ascontiguousarray((m2d[:SC, :SC] * scale).T)
    else:
        maskT = np.ascontiguousarray((m2d * scale).T)
    cosT = np.ascontiguousarray(cos.T, dtype=np.float32)
    sinT = np.ascontiguousarray(sin.T, dtype=np.float32)
    ptT = _round_f32r(_rope_perm_T())
    ones = np.ones((P, 1), dtype=np.float32)

    xT = [_round_f32r(x[b].T) for b in range(B)]

    in_maps = []
    for c in range(N_CORES):
        b = c // (N_CORES // B)
        hg = c % (N_CORES // B)
        rows = slice(hg * HW, (hg + 1) * HW)
        wqkvT = np.concatenate(
            [wq[rows].T, wk[rows].T, wv[rows].T], axis=1
        )
        in_maps.append(
            {
                "xT": xT[b],
                "wqkvT": _round_f32r(wqkvT),
                "woT": _round_f32r(np.ascontiguousarray(wo[:, rows].T)),
                "cosT": cosT,
                "sinT": sinT,
                "PT": ptT,
                "ones": ones,
                "maskT": maskT,
            }
        )

    kw = {}
    if _trace:
        kw = dict(trace=True)
    res = run_bass_kernel_spmd(
        nc, in_maps, core_ids=list(range(N_CORES)), **kw
    )
    global last_exec_time_ns, last_profile_json
    last_exec_time_ns = res.exec_time_ns
    last_profile_json = res.profile_json

    out = np.empty((B, S, D), dtype=np.float32)
    gs = N_CORES // B
    for b in range(B):
        acc = res.results[b * gs]["y"].astype(np.float32).copy()
        for g in range(1, gs):
            acc += res.results[b * gs + g]["y"]
        out[b] = acc
    return out
